# revision 1
# baseline (speedup 1.0000x reference)
"""ARC embeddings kernel for 8 Trainium2 NeuronCores.

Sharding: each core computes the FULL (16-row) token-id scans — scan cost is
sequence-length-bound, not row-bound — but materializes one-hots, matmuls,
LayerNorm and output only for its own 512-token window of T=4096 (T-parallel).
This cuts per-core pos-table traffic 8x vs batch-parallel.

Algorithm:
  - The module is algebraically a table lookup: folding spatial_proj W into
    the tiny row/col tables gives
        x = onehot(ids,row,col).T @ [tok_table ; R@W[:,:256].T ; C@W[:,256:].T] + pos
    -> one K=75 matmul per 128-token tile + an identity matmul accumulating
    pos into the same PSUM bank; LayerNorm on chip.
  - Matmuls run in bf16 at full PE rate with two-term hi/lo splits of the
    tables and pos (the one-hot/identity side is exact in bf16): products
    are exact, PSUM accumulates fp32 -> fp32-grade accuracy.
  - The serial (row, col, in_grid) recurrence of _build_2d_ids is state =
    keep*state + inc, run as a two-level hardware scan: per-chunk
    tensor_tensor_scan (128 lanes x 512) + a tiny inter-chunk affine scan.
"""

import numpy as np

import concourse.bass as bass
import concourse.bacc as bacc
import concourse.tile as tile
from concourse import mybir
from concourse.bass_utils import run_bass_kernel_spmd

B, T, D = 16, 4096, 512
VOCAB, MAX_H, MAX_W = 15, 30, 30
PAD, BOS, EOS, ROW, SEP = 10, 11, 12, 13, 14
LN_EPS = 1e-5
N_CORES = 8
K_EMB = VOCAB + MAX_H + MAX_W  # 75
L = T // N_CORES               # 512: chunk length == per-core token window
CH = T // L                    # 8 chunks per row
RR = B                         # all 16 rows scanned on every core
NTW = L // 128                 # 4 token tiles in the window per row
FP32 = mybir.dt.float32
BF16 = mybir.dt.bfloat16
I32 = mybir.dt.int32
ALU = mybir.AluOpType
ACT_FN = mybir.ActivationFunctionType


def _build_masks(nc, ids_ch, prel):
    """ids_ch: (128,L) int32 -> dict of per-plane (128,L) bf16 mask tiles.
    Separate tiles (not one 3D tile) so the grid scan can start as soon as
    its two inputs are written, not after all mask ops."""
    v = nc.vector
    m = {k: prel.tile([RR * CH, L], BF16, tag=f"m{k}", name=f"m{k}")
         for k in ("kg", "bos", "kr", "row", "kc", "dig")}
    tmp = prel.tile([RR * CH, L], FP32, tag="mtmp")
    tmp2 = prel.tile([RR * CH, L], FP32, tag="mtmp2")

    v.tensor_scalar(out=m["bos"], in0=ids_ch, scalar1=BOS, scalar2=None,
                    op0=ALU.is_equal)
    # is_end = (ids==PAD) + (ids==EOS)
    v.tensor_scalar(out=tmp, in0=ids_ch, scalar1=PAD, scalar2=None, op0=ALU.is_equal)
    v.tensor_scalar(out=tmp2, in0=ids_ch, scalar1=EOS, scalar2=None, op0=ALU.is_equal)
    v.tensor_tensor(out=tmp, in0=tmp, in1=tmp2, op=ALU.add)
    # kg = 1 - (is_bos + is_end)  (grid scan unblocks here)
    v.tensor_tensor(out=tmp, in0=tmp, in1=m["bos"], op=ALU.add)
    v.tensor_scalar(out=m["kg"], in0=tmp, scalar1=-1.0, scalar2=1.0,
                    op0=ALU.mult, op1=ALU.add)
    v.tensor_scalar(out=m["row"], in0=ids_ch, scalar1=ROW, scalar2=None,
                    op0=ALU.is_equal)
    # kr = 1 - (is_bos + is_sep)
    v.tensor_scalar(out=tmp2, in0=ids_ch, scalar1=SEP, scalar2=None, op0=ALU.is_equal)
    v.tensor_tensor(out=tmp2, in0=tmp2, in1=m["bos"], op=ALU.add)
    v.tensor_scalar(out=m["kr"], in0=tmp2, scalar1=-1.0, scalar2=1.0,
                    op0=ALU.mult, op1=ALU.add)
    # kc = kr - is_row
    v.tensor_tensor(out=m["kc"], in0=m["kr"], in1=m["row"], op=ALU.subtract)
    v.tensor_scalar(out=m["dig"], in0=ids_ch, scalar1=9, scalar2=None,
                    op0=ALU.is_le)
    return m


def _recurrence(nc, prel, consts, keep, inc, name):
    """Two-level scan of state[t] = keep[t]*state[t-1] + inc[t] over each row.
    keep/inc: (128, L) chunked views (partition q = row*CH + chunk).
    The (128,1) <-> (16,8) chunk-aggregate reshapes run on the PE via tiny
    selector matmuls (no descriptor-heavy transpose DMAs).
    Returns before (128, L) fp32: state value *before* each position."""
    v = nc.vector
    sel_sb, sel2_sb, mask8_sb = consts
    PQ = prel.tile([RR * CH, 2, L], FP32, tag=f"PQ{name}", name=f"PQ{name}")
    P, Q = PQ[:, 0, :], PQ[:, 1, :]
    v.tensor_tensor_scan(out=Q, data0=keep, data1=inc,
                         initial=0.0, op0=ALU.mult, op1=ALU.add)
    v.tensor_tensor_scan(out=P, data0=keep, data1=keep,
                         initial=1.0, op0=ALU.mult, op1=ALU.bypass)
    # chunk aggregates -> (16, 2, 8) via small transpose DMAs
    inter = prel.tile([RR, 2, CH], FP32, tag=f"I{name}", name=f"I{name}")
    nc.sync.dma_start(out=inter[:, 0, :], in_=P[:, L - 1:L])
    nc.scalar.dma_start(out=inter[:, 1, :], in_=Q[:, L - 1:L])
    # inter-chunk affine scan (length 8 per row)
    S = prel.tile([RR, CH + 1], FP32, tag=f"S{name}", name=f"S{name}")
    v.memset(S[:, 0:1], 0.0)
    v.tensor_tensor_scan(out=S[:, 1:], data0=inter[:, 0, :],
                         data1=inter[:, 1, :],
                         initial=0.0, op0=ALU.mult, op1=ALU.add)
    start = prel.tile([RR * CH, 1], FP32, tag=f"st{name}", name=f"st{name}")
    nc.scalar.dma_start(out=start, in_=S[:, 0:CH])
    # before[q, j] = j==0 ? start_q : Q[q, j-1] + P[q, j-1]*start_q
    bef = prel.tile([RR * CH, L], FP32, tag=f"B{name}", name=f"B{name}")
    v.scalar_tensor_tensor(out=bef[:, 1:L], in0=P[:, 0:L - 1], scalar=start,
                           in1=Q[:, 0:L - 1], op0=ALU.mult, op1=ALU.add)
    v.tensor_copy(out=bef[:, 0:1], in_=start)
    return bef


def _emit_prelude(nc, prel, dpool, consts, iota_sb, ids_d, idsw_d):
    """Token ids -> one-hot matrix ohall (75, 16*L) bf16 for this core's
    T-window (window index = partition id)."""
    ids_ch = prel.tile([RR * CH, L], I32)
    nc.sync.dma_start(
        out=ids_ch,
        in_=ids_d.ap().rearrange("r (c j) -> r c j", j=L))

    m = _build_masks(nc, ids_ch, prel)

    gb = _recurrence(nc, prel, consts, m["kg"], m["bos"], "g")
    rb = _recurrence(nc, prel, consts, m["kr"], m["row"], "r")
    gc_ch = prel.tile([RR * CH, L], BF16)
    nc.vector.tensor_tensor(out=gc_ch, in0=gb, in1=m["dig"], op=ALU.mult)
    cb = _recurrence(nc, prel, consts, m["kc"], gc_ch, "c")

    # emits + combos:  emit = min(gc*before, 29)
    #   combo = (emit + off)*dig + (dig - 1) = (emit + off + 1)*dig - 1
    combo = prel.tile([RR * CH, 2, L], BF16)
    for plane, before, off in ((0, rb, float(VOCAB)),
                               (1, cb, float(VOCAB + MAX_H))):
        em = combo[:, plane, :]
        nc.vector.tensor_tensor(out=em, in0=gc_ch, in1=before, op=ALU.mult)
        nc.vector.tensor_scalar(out=em, in0=em, scalar1=29.0, scalar2=off + 1.0,
                                op0=ALU.min, op1=ALU.add)
        nc.vector.tensor_tensor(out=em, in0=em, in1=m["dig"], op=ALU.mult)
        nc.vector.tensor_scalar(out=em, in0=em, scalar1=1.0, scalar2=None,
                                op0=ALU.subtract)

    # bounce row/col combos to DRAM (full T), broadcast this core's window;
    # the token plane broadcasts straight from the host-provided window ids
    combod = dpool.tile([RR, 2, T], BF16)
    engs = [nc.sync, nc.scalar, nc.gpsimd]
    for m in range(2):
        engs[m].dma_start(
            out=combod[:, m, :].rearrange("r (c j) -> r c j", j=L),
            in_=combo[:, m, :])
    # two independent halves (rows 0-7 / 8-15) so the first half's matmuls
    # can start while the second half's broadcast + compare still run
    HR = RR // 2
    ohs = []
    for h in range(2):
        oh = prel.tile([K_EMB, HR * L], BF16, tag=f"oh{h}", name=f"oh{h}")
        nc.gpsimd.dma_start(
            out=oh[0:VOCAB, :],
            in_=bass.AP(idsw_d, h * HR * L, [[0, VOCAB], [L, HR], [1, L]]))
        for pl, (p0, p1) in enumerate(((VOCAB, VOCAB + MAX_H),
                                       (VOCAB + MAX_H, K_EMB))):
            pid = engs[pl].partition_id()
            ap2 = combod[h * HR:(h + 1) * HR, pl, bass.ts(pid, L)]
            engs[pl].dma_start(
                out=oh[p0:p1, :],
                in_=bass.AP(ap2.tensor, ap2.offset,
                            [[0, p1 - p0]] + list(ap2.ap)))
        # one-hot: 1.0 where combo == partition index (all-bf16 -> 2x mode)
        nc.vector.tensor_scalar(out=oh, in0=oh, scalar1=iota_sb,
                                scalar2=None, op0=ALU.is_equal)
        ohs.append(oh)
    return ohs


def build_program(has_affine: bool, reps: int = 1, mode: str = "full"):
    nc = bacc.Bacc("TRN2", target_bir_lowering=False, debug=False)

    ids_d = nc.dram_tensor("ids", [RR, T], I32, kind="ExternalInput")
    idsw_d = nc.dram_tensor("idsw", [RR, L], BF16, kind="ExternalInput")
    embh_d = nc.dram_tensor("embh", [K_EMB, D], BF16, kind="ExternalInput")
    embl_d = nc.dram_tensor("embl", [K_EMB, D], BF16, kind="ExternalInput")
    poshl_d = nc.dram_tensor("poshl", [L, 2 * D], BF16, kind="ExternalInput")
    ident_d = nc.dram_tensor("ident", [128, 128], BF16, kind="ExternalInput")
    iota_d = nc.dram_tensor("iota75", [K_EMB, 1], FP32, kind="ExternalInput")
    sel_d = nc.dram_tensor("sel", [RR * CH, RR], FP32, kind="ExternalInput")
    sel2_d = nc.dram_tensor("sel2", [RR, RR * CH], FP32, kind="ExternalInput")
    mask8_d = nc.dram_tensor("mask8", [RR * CH, CH], FP32, kind="ExternalInput")
    if has_affine:
        gam_d = nc.dram_tensor("gamma", [1, D], FP32, kind="ExternalInput")
        bet_d = nc.dram_tensor("beta", [1, D], FP32, kind="ExternalInput")
    out_d = nc.dram_tensor("out", [RR, L, D], FP32, kind="ExternalOutput")

    with tile.TileContext(nc) as tc:
        with (
            tc.tile_pool(name="const", bufs=1) as const,
            tc.tile_pool(name="prel", bufs=1) as prel,
            tc.tile_pool(name="dram", bufs=1, space="DRAM") as dpool,
            tc.tile_pool(name="posp", bufs=1) as posp,
            tc.tile_pool(name="psum", bufs=8, space="PSUM") as psump,
            tc.tile_pool(name="stat", bufs=12) as statp,
            tc.tile_pool(name="outp", bufs=6) as outp,
        ):
            # ---------------- constants ----------------
            embh_sb = const.tile([K_EMB, D], BF16)
            nc.sync.dma_start(out=embh_sb, in_=embh_d.ap())
            embl_sb = const.tile([K_EMB, D], BF16)
            nc.sync.dma_start(out=embl_sb, in_=embl_d.ap())
            ident_sb = const.tile([128, 128], BF16)
            nc.sync.dma_start(out=ident_sb, in_=ident_d.ap())
            iota_sb = const.tile([K_EMB, 1], FP32)
            nc.sync.dma_start(out=iota_sb, in_=iota_d.ap())
            eps_sb = const.tile([128, 1], FP32)
            nc.vector.memset(eps_sb, LN_EPS)
            sel_sb = const.tile([RR * CH, RR], FP32)
            nc.sync.dma_start(out=sel_sb, in_=sel_d.ap())
            sel2_sb = const.tile([RR, RR * CH], FP32)
            nc.sync.dma_start(out=sel2_sb, in_=sel2_d.ap())
            mask8_sb = const.tile([RR * CH, CH], FP32)
            nc.sync.dma_start(out=mask8_sb, in_=mask8_d.ap())
            consts = (sel_sb, sel2_sb, mask8_sb)
            if has_affine:
                gam_sb = const.tile([128, D], FP32)
                nc.gpsimd.dma_start(
                    out=gam_sb, in_=bass.AP(gam_d, 0, [[0, 128], [1, D]]))
                bet_sb = const.tile([128, D], FP32)
                nc.gpsimd.dma_start(
                    out=bet_sb, in_=bass.AP(bet_d, 0, [[0, 128], [1, D]]))

            rep_ctx = tc.For_i(0, reps, 1) if reps > 1 else None
            if rep_ctx is not None:
                rep_ctx.__enter__()

            if mode != "main":
                ohs = _emit_prelude(nc, prel, dpool, consts,
                                    iota_sb, ids_d, idsw_d)
            else:
                ohs = [prel.tile([K_EMB, RR * L // 2], BF16, name=f"ohm{h}")
                       for h in range(2)]
                for oh in ohs:
                    nc.vector.memset(oh, 0.25)
            if mode == "prelude":
                ohd = dpool.tile([2, K_EMB, RR * L // 2], BF16)
                for h in range(2):
                    nc.sync.dma_start(out=ohd[h, :, :], in_=ohs[h])

            # ---------------- main loop over rows (window-local) ----------
            if mode != "prelude":
                phl = posp.tile([128, NTW, 2 * D], BF16)
                nc.scalar.dma_start(
                    out=phl,
                    in_=poshl_d.ap().rearrange("(j p) f -> p j f", p=128))
                for r in range(RR):
                    ot = outp.tile([128, NTW, D], FP32, tag="ot")
                    for j in range(NTW):
                        lhs = ohs[r // 8][:, bass.ts((r % 8) * NTW + j, 128)]
                        ps = psump.tile([128, D], FP32, tag="ps")
                        nc.tensor.matmul(ps, lhsT=lhs, rhs=embh_sb,
                                         start=True, stop=False)
                        nc.tensor.matmul(ps, lhsT=lhs, rhs=embl_sb,
                                         start=False, stop=False)
                        nc.tensor.matmul(ps, lhsT=ident_sb, rhs=phl[:, j, 0:D],
                                         start=False, stop=False)
                        nc.tensor.matmul(ps, lhsT=ident_sb,
                                         rhs=phl[:, j, D:2 * D],
                                         start=False, stop=True)

                        st = statp.tile([128, 6], FP32, tag="st")
                        nc.vector.bn_stats(out=st, in_=ps)
                        mv = statp.tile([128, 2], FP32, tag="mv")
                        nc.vector.bn_aggr(out=mv, in_=st)
                        # rstd = 1/sqrt(var + eps)
                        rstd = statp.tile([128, 1], FP32, tag="rstd")
                        nc.scalar.activation(out=rstd, in_=mv[:, 1:2],
                                             func=ACT_FN.Sqrt,
                                             bias=eps_sb, scale=1.0)
                        nc.vector.reciprocal(out=rstd, in_=rstd)
                        # nmr = -mean * rstd
                        nmr = statp.tile([128, 1], FP32, tag="nmr")
                        nc.gpsimd.tensor_scalar(
                            out=nmr, in0=mv[:, 0:1], scalar1=rstd,
                            scalar2=-1.0, op0=ALU.mult, op1=ALU.mult)
                        nc.scalar.activation(out=ot[:, j, :], in_=ps,
                                             func=ACT_FN.Identity,
                                             bias=nmr, scale=rstd)
                        if has_affine:
                            nc.vector.tensor_tensor(
                                out=ot[:, j, :], in0=ot[:, j, :], in1=gam_sb,
                                op=ALU.mult)
                            nc.vector.tensor_tensor(
                                out=ot[:, j, :], in0=ot[:, j, :], in1=bet_sb,
                                op=ALU.add)
                    nc.sync.dma_start(
                        out=out_d.ap()[r, :, :].rearrange(
                            "(j p) d -> p j d", p=128),
                        in_=ot)

            if rep_ctx is not None:
                rep_ctx.__exit__(None, None, None)

    nc.compile()
    return nc


_CACHE: dict[bool, "bass.Bass"] = {}

# test-harness knobs (harmless in production: trace off, result kept for probing)
TRACE = False
TRACE_DIR = None
LAST_RESULT = None


def _split_bf16(x):
    import ml_dtypes
    x = np.asarray(x, np.float32)
    hi = x.astype(ml_dtypes.bfloat16)
    lo = (x - hi.astype(np.float32)).astype(ml_dtypes.bfloat16)
    return hi, lo


def make_in_maps(input_ids, token_table, pos_table, row_table, col_table,
                 w_spatial, ln_gamma, ln_beta):
    import ml_dtypes
    ids = np.ascontiguousarray(np.asarray(input_ids).astype(np.int32))
    tok = np.asarray(token_table, dtype=np.float32)
    pos = np.ascontiguousarray(np.asarray(pos_table, dtype=np.float32)[:T])
    w = np.asarray(w_spatial, dtype=np.float64)
    reff = (np.asarray(row_table, np.float64) @ w[:, :D // 2].T).astype(np.float32)
    ceff = (np.asarray(col_table, np.float64) @ w[:, D // 2:].T).astype(np.float32)
    emb_cat = np.concatenate([tok, reff, ceff], axis=0)
    emb_hi, emb_lo = _split_bf16(emb_cat)
    pos_hi, pos_lo = _split_bf16(pos)
    pos_hl = np.ascontiguousarray(np.concatenate([pos_hi, pos_lo], axis=1))
    gam = np.asarray(ln_gamma, np.float32).reshape(1, D)
    bet = np.asarray(ln_beta, np.float32).reshape(1, D)
    has_affine = not (np.all(gam == 1.0) and np.all(bet == 0.0))

    ident = np.eye(128, dtype=ml_dtypes.bfloat16)
    iota75 = np.arange(K_EMB, dtype=np.float32).reshape(K_EMB, 1)
    q = np.arange(RR * CH)
    sel = (q[:, None] // CH == np.arange(RR)[None, :]).astype(np.float32)
    sel2 = np.ascontiguousarray(sel.T)
    mask8 = (q[:, None] % CH == np.arange(CH)[None, :]).astype(np.float32)
    in_maps = []
    for c in range(N_CORES):
        m = {
            "ids": ids,
            "idsw": np.ascontiguousarray(
                ids[:, c * L:(c + 1) * L].astype(ml_dtypes.bfloat16)),
            "embh": emb_hi, "embl": emb_lo,
            "poshl": np.ascontiguousarray(pos_hl[c * L:(c + 1) * L]),
            "ident": ident, "iota75": iota75,
            "sel": sel, "sel2": sel2, "mask8": mask8,
        }
        if has_affine:
            m["gamma"] = gam
            m["beta"] = bet
        in_maps.append(m)
    return in_maps, has_affine


def kernel(input_ids, token_table, pos_table, row_table, col_table,
           w_spatial, ln_gamma, ln_beta):
    in_maps, has_affine = make_in_maps(
        input_ids, token_table, pos_table, row_table, col_table,
        w_spatial, ln_gamma, ln_beta)

    if has_affine not in _CACHE:
        _CACHE[has_affine] = build_program(has_affine)
    nc = _CACHE[has_affine]

    global LAST_RESULT
    kwargs = {}
    if TRACE:
        kwargs = dict(trace=True, tmpdir=TRACE_DIR)
    res = run_bass_kernel_spmd(nc, in_maps, core_ids=list(range(N_CORES)), **kwargs)
    LAST_RESULT = res
    # cores hold T-windows: concatenate along the token axis
    out = np.concatenate([res.results[c]["out"] for c in range(N_CORES)], axis=1)
    return out.astype(np.float32)



# revision 3
# speedup vs baseline: 1.3607x; 1.3607x over previous
"""ARC embeddings kernel for 8 Trainium2 NeuronCores.

Sharding: each core computes the FULL (16-row) token-id scans -- scan cost is
sequence-length-bound, not row-bound -- but materializes one-hots, matmuls,
LayerNorm and output only for its own 512-token window of T=4096 (T-parallel).

Algorithm:
  - The module is algebraically a table lookup: folding spatial_proj W into
    the tiny row/col tables gives
        x = onehot(ids,row,col).T @ [tok_table ; R@W[:,:256].T ; C@W[:,256:].T] + pos
    -> one K=75 matmul per 128-token tile + an identity matmul accumulating
    pos into the same PSUM bank; LayerNorm on chip.
  - All tables/activations in bf16 (harness tolerance 2e-2 >> bf16 rounding);
    output returned as bf16 and upcast on host.
  - The serial (row, col, in_grid) recurrence of _build_2d_ids is state =
    keep*state + inc, run as a two-level hardware scan: per-chunk
    tensor_tensor_scan (128 lanes x 512, bf16 2x mode) + a tiny inter-chunk
    affine scan. bf16 is exact for the count values that survive the
    min(.,29) clip (integers <= 256 are exact; larger values stay clipped).
"""

import numpy as np

import concourse.bass as bass
import concourse.bacc as bacc
import concourse.tile as tile
from concourse import mybir
from concourse.bass_utils import run_bass_kernel_spmd

B, T, D = 16, 4096, 512
VOCAB, MAX_H, MAX_W = 15, 30, 30
PAD, BOS, EOS, ROW, SEP = 10, 11, 12, 13, 14
LN_EPS = 1e-5
N_CORES = 8
K_EMB = VOCAB + MAX_H + MAX_W  # 75
L = T // N_CORES               # 512: chunk length == per-core token window
CH = T // L                    # 8 chunks per row
RR = B                         # all 16 rows scanned on every core
NTW = L // 128                 # 4 token tiles in the window per row
FP32 = mybir.dt.float32
BF16 = mybir.dt.bfloat16
I32 = mybir.dt.int32
ALU = mybir.AluOpType
ACT_FN = mybir.ActivationFunctionType


def _build_masks(nc, ids_ch, prel):
    """ids_ch: (128,L) bf16 -> dict of per-plane (128,L) bf16 mask tiles."""
    v = nc.vector
    m = {k: prel.tile([RR * CH, L], BF16, tag=f"m{k}", name=f"m{k}")
         for k in ("kg", "bos", "kr", "row", "kc", "dig")}
    tmp = prel.tile([RR * CH, L], BF16, tag="mtmp")
    tmp2 = prel.tile([RR * CH, L], BF16, tag="mtmp2")

    v.tensor_scalar(out=m["bos"], in0=ids_ch, scalar1=float(BOS), scalar2=None,
                    op0=ALU.is_equal)
    # is_end = (ids==PAD) + (ids==EOS)
    v.tensor_scalar(out=tmp, in0=ids_ch, scalar1=float(PAD), scalar2=None,
                    op0=ALU.is_equal)
    v.tensor_scalar(out=tmp2, in0=ids_ch, scalar1=float(EOS), scalar2=None,
                    op0=ALU.is_equal)
    v.tensor_tensor(out=tmp, in0=tmp, in1=tmp2, op=ALU.add)
    # kg = 1 - (is_bos + is_end)  (grid scan unblocks here)
    v.tensor_tensor(out=tmp, in0=tmp, in1=m["bos"], op=ALU.add)
    v.tensor_scalar(out=m["kg"], in0=tmp, scalar1=-1.0, scalar2=1.0,
                    op0=ALU.mult, op1=ALU.add)
    v.tensor_scalar(out=m["row"], in0=ids_ch, scalar1=float(ROW), scalar2=None,
                    op0=ALU.is_equal)
    # kr = 1 - (is_bos + is_sep)
    v.tensor_scalar(out=tmp2, in0=ids_ch, scalar1=float(SEP), scalar2=None,
                    op0=ALU.is_equal)
    v.tensor_tensor(out=tmp2, in0=tmp2, in1=m["bos"], op=ALU.add)
    v.tensor_scalar(out=m["kr"], in0=tmp2, scalar1=-1.0, scalar2=1.0,
                    op0=ALU.mult, op1=ALU.add)
    # kc = kr - is_row
    v.tensor_tensor(out=m["kc"], in0=m["kr"], in1=m["row"], op=ALU.subtract)
    v.tensor_scalar(out=m["dig"], in0=ids_ch, scalar1=9.0, scalar2=None,
                    op0=ALU.is_le)
    return m


def _recurrence(nc, prel, keep, inc, name):
    """Two-level scan of state[t] = keep[t]*state[t-1] + inc[t] over each row.
    keep/inc: (128, L) chunked views (partition q = row*CH + chunk).
    Returns before (128, L) bf16: state value *before* each position."""
    v = nc.vector
    PQ = prel.tile([RR * CH, 2, L], BF16, tag=f"PQ{name}", name=f"PQ{name}")
    P, Q = PQ[:, 0, :], PQ[:, 1, :]
    v.tensor_tensor_scan(out=Q, data0=keep, data1=inc,
                         initial=0.0, op0=ALU.mult, op1=ALU.add)
    v.tensor_tensor_scan(out=P, data0=keep, data1=keep,
                         initial=1.0, op0=ALU.mult, op1=ALU.bypass)
    # chunk aggregates -> (16, 2, 8) via small transpose DMAs
    inter = prel.tile([RR, 2, CH], BF16, tag=f"I{name}", name=f"I{name}")
    nc.sync.dma_start(out=inter[:, 0, :], in_=P[:, L - 1:L])
    nc.scalar.dma_start(out=inter[:, 1, :], in_=Q[:, L - 1:L])
    # inter-chunk affine scan (length 8 per row)
    S = prel.tile([RR, CH + 1], BF16, tag=f"S{name}", name=f"S{name}")
    v.memset(S[:, 0:1], 0.0)
    v.tensor_tensor_scan(out=S[:, 1:], data0=inter[:, 0, :],
                         data1=inter[:, 1, :],
                         initial=0.0, op0=ALU.mult, op1=ALU.add)
    start = prel.tile([RR * CH, 1], BF16, tag=f"st{name}", name=f"st{name}")
    nc.scalar.dma_start(out=start, in_=S[:, 0:CH])
    # before[q, j] = j==0 ? start_q : Q[q, j-1] + P[q, j-1]*start_q
    bef = prel.tile([RR * CH, L], BF16, tag=f"B{name}", name=f"B{name}")
    v.scalar_tensor_tensor(out=bef[:, 1:L], in0=P[:, 0:L - 1], scalar=start,
                           in1=Q[:, 0:L - 1], op0=ALU.mult, op1=ALU.add)
    v.tensor_copy(out=bef[:, 0:1], in_=start)
    return bef


def _emit_prelude(nc, prel, dpool, iota_sb, ids_d, idsw_d):
    """Token ids -> one-hot matrix ohall (75, 16*L) bf16 for this core's
    T-window (window index = partition id)."""
    ids_ch = prel.tile([RR * CH, L], BF16)
    nc.sync.dma_start(
        out=ids_ch,
        in_=ids_d.ap().rearrange("r (c j) -> r c j", j=L))

    m = _build_masks(nc, ids_ch, prel)

    gb = _recurrence(nc, prel, m["kg"], m["bos"], "g")
    gc_ch = prel.tile([RR * CH, L], BF16)
    nc.vector.tensor_tensor(out=gc_ch, in0=gb, in1=m["dig"], op=ALU.mult)
    rb = _recurrence(nc, prel, m["kr"], m["row"], "r")
    cb = _recurrence(nc, prel, m["kc"], gc_ch, "c")

    # emits + combos:  emit = min(gc*before, 29)
    #   combo = (emit + off)*dig + (dig - 1) = (emit + off + 1)*dig - 1
    combo = prel.tile([RR * CH, 2, L], BF16)
    for plane, before, off in ((0, rb, float(VOCAB)),
                               (1, cb, float(VOCAB + MAX_H))):
        em = combo[:, plane, :]
        nc.vector.tensor_tensor(out=em, in0=gc_ch, in1=before, op=ALU.mult)
        nc.vector.tensor_scalar(out=em, in0=em, scalar1=29.0, scalar2=off + 1.0,
                                op0=ALU.min, op1=ALU.add)
        nc.vector.tensor_tensor(out=em, in0=em, in1=m["dig"], op=ALU.mult)
        nc.vector.tensor_scalar(out=em, in0=em, scalar1=1.0, scalar2=None,
                                op0=ALU.subtract)

    # bounce row/col combos to DRAM (full T), broadcast this core's window;
    # the token plane broadcasts straight from the host-provided window ids
    combod = dpool.tile([RR, 2, T], BF16)
    engs = [nc.sync, nc.scalar, nc.gpsimd]
    for mm in range(2):
        engs[mm].dma_start(
            out=combod[:, mm, :].rearrange("r (c j) -> r c j", j=L),
            in_=combo[:, mm, :])
    # two independent halves (rows 0-7 / 8-15) so the first half's matmuls
    # can start while the second half's broadcast + compare still run
    HR = RR // 2
    ohs = []
    for h in range(2):
        oh = prel.tile([K_EMB, HR * L], BF16, tag=f"oh{h}", name=f"oh{h}")
        nc.gpsimd.dma_start(
            out=oh[0:VOCAB, :],
            in_=bass.AP(idsw_d, h * HR * L, [[0, VOCAB], [L, HR], [1, L]]))
        for pl, (p0, p1) in enumerate(((VOCAB, VOCAB + MAX_H),
                                       (VOCAB + MAX_H, K_EMB))):
            pid = engs[pl].partition_id()
            ap2 = combod[h * HR:(h + 1) * HR, pl, bass.ts(pid, L)]
            engs[pl].dma_start(
                out=oh[p0:p1, :],
                in_=bass.AP(ap2.tensor, ap2.offset,
                            [[0, p1 - p0]] + list(ap2.ap)))
        # one-hot: 1.0 where combo == partition index (all-bf16 -> fast mode)
        nc.vector.tensor_scalar(out=oh, in0=oh, scalar1=iota_sb,
                                scalar2=None, op0=ALU.is_equal)
        ohs.append(oh)
    return ohs


def build_program(has_affine: bool, reps: int = 1):
    nc = bacc.Bacc("TRN2", target_bir_lowering=False, debug=False)

    ids_d = nc.dram_tensor("ids", [RR, T], BF16, kind="ExternalInput")
    idsw_d = nc.dram_tensor("idsw", [RR, L], BF16, kind="ExternalInput")
    emb_d = nc.dram_tensor("emb", [K_EMB, D], BF16, kind="ExternalInput")
    posw_d = nc.dram_tensor("posw", [L, D], BF16, kind="ExternalInput")
    ident_d = nc.dram_tensor("ident", [128, 128], BF16, kind="ExternalInput")
    iota_d = nc.dram_tensor("iota75", [K_EMB, 1], FP32, kind="ExternalInput")
    if has_affine:
        gam_d = nc.dram_tensor("gamma", [1, D], FP32, kind="ExternalInput")
        bet_d = nc.dram_tensor("beta", [1, D], FP32, kind="ExternalInput")
    out_d = nc.dram_tensor("out", [RR, NTW, 128, D], BF16,
                           kind="ExternalOutput")

    with tile.TileContext(nc) as tc:
        with (
            tc.tile_pool(name="const", bufs=1) as const,
            tc.tile_pool(name="prel", bufs=1) as prel,
            tc.tile_pool(name="dram", bufs=1, space="DRAM") as dpool,
            tc.tile_pool(name="psum", bufs=8, space="PSUM") as psump,
            tc.tile_pool(name="stat", bufs=12) as statp,
            tc.tile_pool(name="outp", bufs=4) as outp,
        ):
            # ---------------- constants ----------------
            emb_sb = const.tile([K_EMB, D], BF16)
            nc.sync.dma_start(out=emb_sb, in_=emb_d.ap())
            ident_sb = const.tile([128, 128], BF16)
            nc.sync.dma_start(out=ident_sb, in_=ident_d.ap())
            iota_sb = const.tile([K_EMB, 1], FP32)
            nc.sync.dma_start(out=iota_sb, in_=iota_d.ap())
            eps_sb = const.tile([128, 1], FP32)
            nc.vector.memset(eps_sb, LN_EPS)
            posw_sb = const.tile([128, NTW, D], BF16)
            nc.scalar.dma_start(
                out=posw_sb,
                in_=posw_d.ap().rearrange("(j p) f -> p j f", p=128))
            if has_affine:
                gam_sb = const.tile([128, D], FP32)
                nc.gpsimd.dma_start(
                    out=gam_sb, in_=bass.AP(gam_d, 0, [[0, 128], [1, D]]))
                bet_sb = const.tile([128, D], FP32)
                nc.gpsimd.dma_start(
                    out=bet_sb, in_=bass.AP(bet_d, 0, [[0, 128], [1, D]]))

            rep_ctx = tc.For_i(0, reps, 1) if reps > 1 else None
            if rep_ctx is not None:
                rep_ctx.__enter__()

            ohs = _emit_prelude(nc, prel, dpool, iota_sb, ids_d, idsw_d)

            # ---------------- main loop over rows (window-local) ----------
            for r in range(RR):
                ot = outp.tile([128, NTW, D], BF16, tag="ot")
                mv = statp.tile([128, NTW, 2], FP32, tag="mv")
                pss = []
                for j in range(NTW):
                    lhs = ohs[r // 8][:, bass.ts((r % 8) * NTW + j, 128)]
                    ps = psump.tile([128, D], FP32, tag="ps")
                    pss.append(ps)
                    nc.tensor.matmul(ps, lhsT=lhs, rhs=emb_sb,
                                     start=True, stop=False)
                    nc.tensor.matmul(ps, lhsT=ident_sb, rhs=posw_sb[:, j, :],
                                     start=False, stop=True)
                    st = statp.tile([128, 6], FP32, tag="st")
                    nc.vector.bn_stats(out=st, in_=ps)
                    nc.vector.bn_aggr(out=mv[:, j, :], in_=st)
                # rstd = 1/sqrt(var + eps), batched over the row's 4 tiles
                rstd = statp.tile([128, NTW], FP32, tag="rstd")
                nc.scalar.activation(out=rstd, in_=mv[:, :, 1],
                                     func=ACT_FN.Sqrt, bias=eps_sb, scale=1.0)
                nc.vector.reciprocal(out=rstd, in_=rstd)
                # nmr = -mean * rstd
                nmr = statp.tile([128, NTW], FP32, tag="nmr")
                nc.vector.scalar_tensor_tensor(
                    out=nmr, in0=mv[:, :, 0], scalar=-1.0, in1=rstd,
                    op0=ALU.mult, op1=ALU.mult)
                for j in range(NTW):
                    nc.scalar.activation(out=ot[:, j, :], in_=pss[j],
                                         func=ACT_FN.Identity,
                                         bias=nmr[:, j:j + 1],
                                         scale=rstd[:, j:j + 1])
                    if has_affine:
                        nc.vector.tensor_tensor(
                            out=ot[:, j, :], in0=ot[:, j, :], in1=gam_sb,
                            op=ALU.mult)
                        nc.vector.tensor_tensor(
                            out=ot[:, j, :], in0=ot[:, j, :], in1=bet_sb,
                            op=ALU.add)
                nc.sync.dma_start(
                    out=out_d.ap()[r].rearrange("j p d -> p j d"),
                    in_=ot)

            if rep_ctx is not None:
                rep_ctx.__exit__(None, None, None)

    nc.compile()
    return nc


_CACHE: dict[bool, "bass.Bass"] = {}

# test-harness knobs (harmless in production: trace off, result kept)
TRACE = False
TRACE_DIR = None
LAST_RESULT = None


def make_in_maps(input_ids, token_table, pos_table, row_table, col_table,
                 w_spatial, ln_gamma, ln_beta):
    import ml_dtypes
    ids = np.asarray(input_ids).astype(np.float32)
    idsb = np.ascontiguousarray(ids.astype(ml_dtypes.bfloat16))
    tok = np.asarray(token_table, dtype=np.float64)
    pos = np.ascontiguousarray(np.asarray(pos_table, dtype=np.float32)[:T])
    w = np.asarray(w_spatial, dtype=np.float64)
    reff = row_table.astype(np.float64) @ w[:, :D // 2].T
    ceff = col_table.astype(np.float64) @ w[:, D // 2:].T
    emb_cat = np.concatenate([tok, reff, ceff], axis=0).astype(np.float32)
    emb_bf = np.ascontiguousarray(emb_cat.astype(ml_dtypes.bfloat16))
    pos_bf = pos.astype(ml_dtypes.bfloat16)
    gam = np.asarray(ln_gamma, np.float32).reshape(1, D)
    bet = np.asarray(ln_beta, np.float32).reshape(1, D)
    has_affine = not (np.all(gam == 1.0) and np.all(bet == 0.0))

    ident = np.eye(128, dtype=ml_dtypes.bfloat16)
    iota75 = np.arange(K_EMB, dtype=np.float32).reshape(K_EMB, 1)
    in_maps = []
    for c in range(N_CORES):
        m = {
            "ids": idsb,
            "idsw": np.ascontiguousarray(idsb[:, c * L:(c + 1) * L]),
            "emb": emb_bf,
            "posw": np.ascontiguousarray(pos_bf[c * L:(c + 1) * L]),
            "ident": ident, "iota75": iota75,
        }
        if has_affine:
            m["gamma"] = gam
            m["beta"] = bet
        in_maps.append(m)
    return in_maps, has_affine


def kernel(input_ids, token_table, pos_table, row_table, col_table,
           w_spatial, ln_gamma, ln_beta):
    in_maps, has_affine = make_in_maps(
        input_ids, token_table, pos_table, row_table, col_table,
        w_spatial, ln_gamma, ln_beta)

    if has_affine not in _CACHE:
        _CACHE[has_affine] = build_program(has_affine)
    nc = _CACHE[has_affine]

    global LAST_RESULT
    kwargs = {}
    if TRACE:
        kwargs = dict(trace=True, tmpdir=TRACE_DIR)
    res = run_bass_kernel_spmd(nc, in_maps, core_ids=list(range(N_CORES)),
                               **kwargs)
    LAST_RESULT = res
    # core c holds (16, NTW, 128, D) bf16 for tokens [c*L, (c+1)*L)
    outs = [np.asarray(res.results[c]["out"]).reshape(RR, L, D)
            for c in range(N_CORES)]
    return np.concatenate(outs, axis=1).astype(np.float32)
